# revision 1
# baseline (speedup 1.0000x reference)
"""Multi-head self-attention (pre-LN, residual) Trainium2 Bass kernel.

Problem: B=4, S=2048, D=128, H=4, Dh=32, fp32 -> rel err ~1.2e-3.
Sharding: 8 cores = 4 batches x 2 query-halves (1024 queries/core).
Each core receives its batch's full x, row-shuffled by the host so that
(a) the core's query half occupies device positions 0..1023 (attention is
permutation-invariant over keys) and (b) each SBUF partition loads
consecutive DRAM rows (8KB-contiguous DMA chunks at full bandwidth).

Fully transposed dataflow ([feature, seq] layouts) so the softmax
reduction rides the PE and no giant P-matrix transpose is needed:
  xn0^T --W--> Q^T,K^T [hd, s] bf16;  V [s, hd] bf16
  S^T[k,q] = K^T.T @ Q^T     2+2 heads packed via PE row-tiling (K=32)
  P_A = exp(S^T - 8)         heads {0,2} on ACT (table exp, bf16 out)
  P_B = schraudolph(S^T - 8) heads {1,3} on DVE: ONE tensor_scalar
                             (x*SA+SB) with int16 convert-on-write whose
                             bits are bf16 exp (min-RMS corrected, ~2%)
  ctx^T[hd,q] = V.T @ P      4 heads packed via PE col-tiling (M=32)
  den[hd,q]   = 1.T @ P      col-tiled ones-matmul (per-head row blocks)
  out^T = Wo.T @ (ctx^T * recip_approx(den)) + (x^T + bias)
gamma/beta/all biases are folded into projection weights / per-partition
bias columns.  QKV/out projections run as float32r (tf32-like); scores
and P-side matmuls in bf16; all PSUM accumulation fp32.

Scheduling: LN/transpose/projection prep is emitted in 4-tile blocks
interleaved with the attention k-loop (blocks 2,3 inject into chunk 0);
scores+exps are emitted one ktile ahead of ctx/den so the in-order PE
stream always has runnable work while exps are in flight; softmax recip
uses the custom-DVE fast reciprocal (~51 ULP); dummy full-array matmul
bursts warm the PE HAM clock-gate (tile_position'd matmuls alone do not
hold it at 2.4 GHz).
"""

import sys

if "/opt/trn_rl_repo" not in sys.path:
    sys.path.insert(0, "/opt/trn_rl_repo")

import numpy as np

import concourse.bacc as bacc
import concourse.tile as tile
import concourse.mybir as mybir
from concourse.bass_utils import run_bass_kernel_spmd
from concourse.masks import make_identity

F32 = mybir.dt.float32
F32R = mybir.dt.float32r
BF16 = mybir.dt.bfloat16
I16 = mybir.dt.int16
AF = mybir.ActivationFunctionType
OP = mybir.AluOpType

B, S, D = 4, 2048, 128
H, DH = 4, 32
N_CORES = 8
QH = S // 2  # queries per core
NT = S // 128  # 16 s-tiles
NQT = QH // 128  # 8 q-tiles
CHUNK = 512
NCH = QH // CHUNK  # q-chunks per core
NKT = S // 128  # k-tiles
EPS = 1e-6
SHIFT = 8.0
ISQ = 1.0 / np.sqrt(np.float32(DH))
# Schraudolph bf16 exp: int16(x*SA + SB).bits == bf16(exp(x - SHIFT))
SA = float(128.0 / np.log(2.0))
SB = float(127.0 * 128.0 - 0.0579 * 128.0 - SHIFT * 128.0 / np.log(2.0))

GROUPS = ((0, 2), (1, 3))  # (A on ACT, B on DVE); same-parity heads share
# a ctxden bank so Wo row masks stay partition-aligned.

_compiled = None


def _build():
    nc = bacc.Bacc(
        "TRN2",
        target_bir_lowering=False,
        debug=False,
        enable_asserts=False,
        num_devices=N_CORES,
    )

    xkv_d = nc.dram_tensor("xkv", [S, D], F32, kind="ExternalInput").ap()
    wq_d = nc.dram_tensor("wq", [D, D], F32, kind="ExternalInput").ap()
    wk_d = nc.dram_tensor("wk", [D, D], F32, kind="ExternalInput").ap()
    wv_d = nc.dram_tensor("wv", [D, D], F32, kind="ExternalInput").ap()
    wo_d = nc.dram_tensor("wo", [D, D], F32, kind="ExternalInput").ap()
    # rows: gamma, beta, bq, bk, bv, bo
    vecs_d = nc.dram_tensor("vecs", [6, D], F32, kind="ExternalInput").ap()
    outT_d = nc.dram_tensor("outT", [D, QH], F32, kind="ExternalOutput").ap()

    with tile.TileContext(nc) as tc:
        consts = tc.alloc_tile_pool(name="consts", bufs=1)
        sbW = tc.alloc_tile_pool(name="sbW", bufs=1)
        sbBig = tc.alloc_tile_pool(name="sbBig", bufs=1)
        sbTmp = tc.alloc_tile_pool(name="sbTmp", bufs=3)

        ident = consts.tile([128, 128], F32)
        make_identity(nc, ident)
        nshift = consts.tile([128, 1], F32)
        nc.vector.memset(nshift, -SHIFT)
        epsc = consts.tile([128, 1], F32)
        nc.vector.memset(epsc, EPS)
        zeroc = consts.tile([128, 1], F32)
        nc.vector.memset(zeroc, 0.0)
        wsrc = consts.tile([128, 512], BF16)
        nc.vector.memset(wsrc, 0.5)
        wones = consts.tile([128, DH], BF16)
        nc.vector.memset(wones, 1.0)

        # ---- input DMAs ----
        wq_raw = sbW.tile([D, D], F32)
        wk_raw = sbW.tile([D, D], F32)
        wv_raw = sbW.tile([D, D], F32)
        wo_raw = sbW.tile([D, D], F32)
        nc.scalar.dma_start(out=wq_raw, in_=wq_d)
        nc.scalar.dma_start(out=wk_raw, in_=wk_d)
        nc.scalar.dma_start(out=wv_raw, in_=wv_d)
        nc.scalar.dma_start(out=wo_raw, in_=wo_d)
        smallT = sbW.tile([D, 6], F32)  # cols: gamma,beta,bq,bk,bv,bo
        nc.scalar.dma_start(out=smallT, in_=vecs_d.rearrange("v d -> d v"))

        xkv_sb = sbBig.tile([128, NT, 128], F32)
        xkv_r = xkv_d.rearrange("(p t) d -> p t d", t=NT)
        for c4 in range(4):
            nc.sync.dma_start(
                out=xkv_sb[:, c4 * 4 : (c4 + 1) * 4, :],
                in_=xkv_r[:, c4 * 4 : (c4 + 1) * 4, :],
            )

        ps_a = tc.alloc_tile_pool(name="ps_a", bufs=2, space="PSUM")

        # HAM warm-up chain (independent; fills PE during DVE/DMA setup)
        for _ in range(8):
            wps = ps_a.tile([128, 512], F32, name="wps", tag="a")
            nc.tensor.matmul(wps[0:DH, :], wones, wsrc, start=True, stop=True)

        # ---- fold gamma/beta/biases ----
        gam = smallT[:, 0:1]
        bet = smallT[:, 1:2]
        gq = sbW.tile([128, 1], F32)
        nc.vector.tensor_scalar_mul(gq, gam, float(ISQ))
        wq_f = sbW.tile([D, D], F32R)
        wk_f = sbW.tile([D, D], F32R)
        wv_f = sbW.tile([D, D], F32R)
        nc.vector.tensor_scalar_mul(wq_f, wq_raw, gq)
        nc.vector.tensor_scalar_mul(wk_f, wk_raw, gam)
        nc.vector.tensor_scalar_mul(wv_f, wv_raw, gam)

        wo_r = sbW.tile([D, D], F32R)
        nc.vector.tensor_copy(wo_r, wo_raw)
        bqe = sbW.tile([128, 1], F32)
        bke = sbW.tile([128, 1], F32)
        bve = sbW.tile([128, 1], F32)
        rbias = sbW.tile([128, 1], F32)
        t_ps = ps_a.tile([128, 1], F32, tag="a")
        nc.tensor.matmul(t_ps, wq_raw, bet, start=True, stop=True)
        nc.vector.tensor_scalar(
            bqe, t_ps, smallT[:, 2:3], float(ISQ), op0=OP.add, op1=OP.mult
        )
        t_ps = ps_a.tile([128, 1], F32, tag="a")
        nc.tensor.matmul(t_ps, wk_raw, bet, start=True, stop=True)
        nc.vector.tensor_scalar_add(bke, t_ps, smallT[:, 3:4])
        t_ps = ps_a.tile([128, 1], F32, tag="a")
        nc.tensor.matmul(t_ps, wv_raw, bet, start=True, stop=True)
        nc.vector.tensor_scalar_add(bve, t_ps, smallT[:, 4:5])
        t_ps = ps_a.tile([128, 1], F32, tag="a")
        nc.tensor.matmul(t_ps, wo_raw, bve, start=True, stop=True)
        nc.vector.tensor_scalar_add(rbias, t_ps, smallT[:, 5:6])


        # ---- LayerNorm + transposes + projections, pipelined with the
        # attention loop: prep block b covers s-tiles 4b..4b+3 (their LN,
        # transpose, K/Q projection chunk and V tiles); attention ktiles
        # 4b..4b+3 of chunk 0 only need blocks <= b, so emission interleaves
        # prep blocks with attention ktiles and the PE stream never waits on
        # the full setup.
        mv_all = sbBig.tile([128, NT, 2], F32)
        lnv = sbBig.tile([128, NT], F32)
        rs_all = sbBig.tile([128, NT], F32)
        bias2 = sbBig.tile([128, NT], F32)
        xn0_sb = sbBig.tile([128, NT, 128], F32)
        xkvT = sbBig.tile([128, S], F32R)  # xn0^T [d, s]
        kT = sbBig.tile([128, S], BF16)
        qT = sbBig.tile([128, QH], BF16)
        v_sb = sbBig.tile([128, NT, 128], BF16)
        residT = sbBig.tile([128, QH], F32)  # x^T + resid_bias (query half)

        def prep_block(b4):
            for t in range(b4 * 4, b4 * 4 + 4):
                stats = sbTmp.tile([128, 6], F32, tag="st")
                nc.vector.bn_stats(stats, xkv_sb[:, t, :])
                nc.vector.bn_aggr(mv_all[:, t, :], stats)
            sl4 = slice(b4 * 4, b4 * 4 + 4)
            # rs = sqrt(1/(var+eps)): reciprocal exact on DVE, Sqrt on ACT
            nc.vector.tensor_scalar_add(lnv[:, sl4], mv_all[:, sl4, 1], epsc)
            nc.vector.reciprocal(bias2[:, sl4], lnv[:, sl4])
            nc.scalar.activation(
                rs_all[:, sl4], bias2[:, sl4], AF.Sqrt, bias=zeroc, scale=1.0
            )
            for t in range(b4 * 4, b4 * 4 + 4):
                nc.vector.tensor_scalar(
                    xn0_sb[:, t, :],
                    xkv_sb[:, t, :],
                    mv_all[:, t, 0:1],
                    rs_all[:, t : t + 1],
                    op0=OP.subtract,
                    op1=OP.mult,
                )
                tp = ps_a.tile([128, 128], F32, tag="a")
                nc.tensor.transpose(tp, xn0_sb[:, t, :], ident)
                nc.scalar.copy(xkvT[:, t * 128 : (t + 1) * 128], tp)
            c = b4
            pp = ps_a.tile([128, CHUNK], F32, tag="a")
            nc.tensor.matmul(
                pp, wk_f, xkvT[:, c * CHUNK : (c + 1) * CHUNK], start=True, stop=True
            )
            nc.vector.tensor_scalar_add(kT[:, c * CHUNK : (c + 1) * CHUNK], pp, bke)
            if c < NCH:
                pp = ps_a.tile([128, CHUNK], F32, tag="a")
                nc.tensor.matmul(
                    pp, wq_f, xkvT[:, c * CHUNK : (c + 1) * CHUNK],
                    start=True, stop=True,
                )
                nc.vector.tensor_scalar_add(
                    qT[:, c * CHUNK : (c + 1) * CHUNK], pp, bqe
                )
            for t in range(b4 * 4, b4 * 4 + 4):
                pp = ps_a.tile([128, 128], F32, tag="a")
                nc.tensor.matmul(
                    pp, xkvT[:, t * 128 : (t + 1) * 128], wv_f, start=True, stop=True
                )
                nc.scalar.copy(v_sb[:, t, :], pp)

        def resid_block(ts_range):
            for t in ts_range:
                tp = ps_a.tile([128, 128], F32, tag="a")
                nc.tensor.transpose(tp, xkv_sb[:, t, :], ident)
                nc.vector.tensor_scalar_add(
                    residT[:, t * 128 : (t + 1) * 128], tp, rbias
                )

        # ---- attention (interleaved with prep blocks) ----
        ps_e = tc.alloc_tile_pool(name="ps_e", bufs=1, space="PSUM")
        pPool = tc.alloc_tile_pool(name="pPool", bufs=6)

        ctx_sb = sbBig.tile([128, NCH, CHUNK], F32)
        den_all = sbBig.tile([128, NCH, CHUNK], F32)
        ctx_ps = None
        den_ps = None

        def attn_scores(qc, kt):
            q0 = qc * CHUNK
            k0 = kt * 128
            p_sb = [None, None]
            for g, heads in enumerate(GROUPS):
                sp = ps_e.tile([128, 2 * CHUNK], F32, name=f"s{g}", tag="s", bufs=2)
                for i, h in enumerate(heads):
                    nc.tensor.matmul(
                        sp[:, i * CHUNK : (i + 1) * CHUNK],
                        kT[h * DH : (h + 1) * DH, k0 : k0 + 128],
                        qT[h * DH : (h + 1) * DH, q0 : q0 + CHUNK],
                        start=True,
                        stop=True,
                        tile_position=(h * DH, 0),
                    )
                if g == 0:
                    pA = pPool.tile([128, 2 * CHUNK], BF16, tag="p")
                    nc.scalar.activation(pA, sp, AF.Exp, bias=nshift, scale=1.0)
                    p_sb[0] = pA
                else:
                    pB = pPool.tile([128, 2 * CHUNK], I16, tag="p")
                    nc.vector.tensor_scalar(pB, sp, SA, SB, op0=OP.mult, op1=OP.add)
                    p_sb[1] = pB.bitcast(BF16)
            return p_sb

        def attn_ctxden(qc, kt, p_sb):
            for g, heads in enumerate(GROUPS):
                for i, h in enumerate(heads):
                    nc.tensor.matmul(
                        ctx_ps[h * DH : (h + 1) * DH, :],
                        v_sb[:, kt, h * DH : (h + 1) * DH],
                        p_sb[g][:, i * CHUNK : (i + 1) * CHUNK],
                        start=(kt == 0),
                        stop=(kt == NKT - 1),
                        tile_position=(0, h * DH),
                    )
            for g, heads in enumerate(GROUPS):
                for i, h in enumerate(heads):
                    nc.tensor.matmul(
                        den_ps[h * DH : (h + 1) * DH, :],
                        wones,
                        p_sb[g][:, i * CHUNK : (i + 1) * CHUNK],
                        start=(kt == 0),
                        stop=(kt == NKT - 1),
                        tile_position=(0, h * DH),
                    )

        prep_block(0)
        resid_block(range(0, 4))
        prep_block(1)
        resid_block(range(4, NQT))
        for _ in range(6):
            warm2 = ps_a.tile([128, CHUNK], F32, name="warm2", tag="a")
            nc.tensor.matmul(warm2[0:DH, :], wones, wsrc, start=True, stop=True)

        def chunk_tail(qc):
            q0 = qc * CHUNK
            ctxn = sbTmp.tile([128, CHUNK], F32R, tag="cn")
            nc.vector.tensor_mul(ctxn, ctx_sb[:, qc, :], den_all[:, qc, :])
            out_ps = ps_a.tile([128, CHUNK], F32, name="out_ps", tag="a")
            nc.tensor.matmul(out_ps, wo_r, ctxn, start=True, stop=True)
            fin = sbTmp.tile([128, CHUNK], F32, tag="fin")
            nc.vector.tensor_add(fin, out_ps, residT[:, q0 : q0 + CHUNK])
            nc.sync.dma_start(out=outT_d[:, q0 : q0 + CHUNK], in_=fin)

        # chunk 0 interleaved with remaining prep; scores/exp emitted one
        # ktile ahead of ctx/den so the in-order PE stream always has
        # runnable work while the exps of the previous ktile are in flight.
        ctx_ps = ps_e.tile([128, CHUNK], F32, name="ctx0", tag="ctx")
        den_ps = ps_e.tile([128, CHUNK], F32, name="den0", tag="den")
        pending = attn_scores(0, 0)
        for kt in range(NKT):
            if kt == 3:
                prep_block(2)
            elif kt == 7:
                prep_block(3)
            nxt = attn_scores(0, kt + 1) if kt + 1 < NKT else None
            attn_ctxden(0, kt, pending)
            pending = nxt
        nc.vector.tensor_copy(ctx_sb[:, 0, :], ctx_ps)
        nc.vector.reciprocal_approx_fast(den_all[:, 0, :], den_ps)

        # chunk 1
        ctx_ps = ps_e.tile([128, CHUNK], F32, name="ctx1", tag="ctx")
        den_ps = ps_e.tile([128, CHUNK], F32, name="den1", tag="den")
        for _ in range(4):
            warm3 = ps_a.tile([128, CHUNK], F32, name="warm3", tag="a")
            nc.tensor.matmul(warm3[0:DH, :], wones, wsrc, start=True, stop=True)
        pending = attn_scores(1, 0)
        for kt in range(NKT):
            nxt = attn_scores(1, kt + 1) if kt + 1 < NKT else None
            attn_ctxden(1, kt, pending)
            pending = nxt
        nc.vector.tensor_copy(ctx_sb[:, 1, :], ctx_ps)
        nc.vector.reciprocal_approx_fast(den_all[:, 1, :], den_ps)
        chunk_tail(0)
        chunk_tail(1)

        pPool.release()
        ps_e.release()
        ps_a.release()
        sbTmp.release()
        sbBig.release()
        sbW.release()
        consts.release()

    nc.compile()
    return nc


def _get_compiled():
    global _compiled
    if _compiled is None:
        _compiled = _build()
    return _compiled


# device position j <- host row (j%128)*16 + j//128
_DEV2HOST = (np.arange(S) % 128) * NT + np.arange(S) // 128
_HOSTPERM = np.empty(S, dtype=np.int64)
_HOSTPERM[_DEV2HOST] = np.arange(S)


def kernel(x, Wq, bq, Wk, bk, Wv, bv, gamma, beta, Wo, bo):
    x = np.asarray(x, dtype=np.float32)
    vecs = np.stack(
        [np.asarray(a, dtype=np.float32) for a in (gamma, beta, bq, bk, bv, bo)]
    )
    wq = np.ascontiguousarray(np.asarray(Wq, dtype=np.float32))
    wk = np.ascontiguousarray(np.asarray(Wk, dtype=np.float32))
    wv = np.ascontiguousarray(np.asarray(Wv, dtype=np.float32))
    wo = np.ascontiguousarray(np.asarray(Wo, dtype=np.float32))

    nc = _get_compiled()

    in_maps = []
    for c in range(N_CORES):
        b, half = c // 2, c % 2
        off = half * QH
        xroll = np.roll(x[b], -off, axis=0)
        xin = np.ascontiguousarray(xroll[_HOSTPERM])
        in_maps.append(
            {"xkv": xin, "wq": wq, "wk": wk, "wv": wv, "wo": wo, "vecs": vecs}
        )

    res = run_bass_kernel_spmd(nc, in_maps, core_ids=list(range(N_CORES)), trace=False)

    out = np.empty((B, S, D), dtype=np.float32)
    for c in range(N_CORES):
        b, half = c // 2, c % 2
        off = half * QH
        out[b, off : off + QH, :] = res.results[c]["outT"].T
    return out



# revision 14
# speedup vs baseline: 1.0602x; 1.0602x over previous
"""Multi-head self-attention (pre-LN, residual) Trainium2 Bass kernel.

Problem: B=4, S=2048, D=128, H=4, Dh=32, fp32 -> rel err ~1.2e-3.
Sharding: 8 cores = 4 batches x 2 query-halves (1024 queries/core).
Each core receives its batch's full x, row-shuffled by the host so that
(a) the core's query half occupies device positions 0..1023 (attention is
permutation-invariant over keys) and (b) each SBUF partition loads
consecutive DRAM rows (8KB-contiguous DMA chunks at full bandwidth).

Fully transposed dataflow ([feature, seq] layouts) so the softmax
reduction rides the PE and no giant P-matrix transpose is needed:
  xn0^T --W--> Q^T,K^T [hd, s] bf16;  V [s, hd] bf16
  S^T[k,q] = K^T.T @ Q^T     2+2 heads packed via PE row-tiling (K=32)
  P_A = exp(S^T - 8)         heads {0,2} on ACT (table exp, bf16 out)
  P_B = schraudolph(S^T - 8) heads {1,3} on DVE: ONE tensor_scalar
                             (x*SA+SB) with int16 convert-on-write whose
                             bits are bf16 exp (min-RMS corrected, ~2%)
  ctx^T[hd,q] = V.T @ P      4 heads packed via PE col-tiling (M=32)
  den[hd,q]   = 1.T @ P      col-tiled ones-matmul (per-head row blocks)
  out^T = Wo.T @ (ctx^T * recip_approx(den)) + (x^T + bias)
gamma/beta/all biases are folded into projection weights / per-partition
bias columns.  QKV/out projections run as float32r (tf32-like); scores
and P-side matmuls in bf16; all PSUM accumulation fp32.

Scheduling: LN/transpose/projection prep is emitted in 4-tile blocks
interleaved with the attention k-loop (blocks 2,3 inject into chunk 0);
scores+exps are emitted one ktile ahead of ctx/den so the in-order PE
stream always has runnable work while exps are in flight; softmax recip
uses the custom-DVE fast reciprocal (~51 ULP); dummy full-array matmul
bursts warm the PE HAM clock-gate (tile_position'd matmuls alone do not
hold it at 2.4 GHz).
"""

import sys

if "/opt/trn_rl_repo" not in sys.path:
    sys.path.insert(0, "/opt/trn_rl_repo")

import numpy as np

import concourse.bacc as bacc
import concourse.tile as tile
import concourse.mybir as mybir
from concourse.bass_utils import run_bass_kernel_spmd
from concourse.masks import make_identity

F32 = mybir.dt.float32
F32R = mybir.dt.float32r
BF16 = mybir.dt.bfloat16
I16 = mybir.dt.int16
AF = mybir.ActivationFunctionType
OP = mybir.AluOpType

B, S, D = 4, 2048, 128
H, DH = 4, 32
N_CORES = 8
QH = S // 2  # queries per core
NT = S // 128  # 16 s-tiles
NQT = QH // 128  # 8 q-tiles
CHUNK = 512
NCH = QH // CHUNK  # q-chunks per core
NKT = S // 128  # k-tiles
EPS = 1e-6
SHIFT = 8.0
ISQ = 1.0 / np.sqrt(np.float32(DH))
# Schraudolph bf16 exp: int16(x*SA + SB).bits == bf16(exp(x - SHIFT))
SA = float(128.0 / np.log(2.0))
SB = float(127.0 * 128.0 - 0.0579 * 128.0 - SHIFT * 128.0 / np.log(2.0))

GROUPS = ((0, 2), (1, 3))  # (A on ACT, B on DVE); same-parity heads share
# a ctxden bank so Wo row masks stay partition-aligned.

_compiled = None


def _build():
    nc = bacc.Bacc(
        "TRN2",
        target_bir_lowering=False,
        debug=False,
        enable_asserts=False,
        num_devices=N_CORES,
    )

    xkv_d = nc.dram_tensor("xkv", [S, D], F32, kind="ExternalInput").ap()
    xt_d = nc.dram_tensor("xt", [D, QH], F32, kind="ExternalInput").ap()
    wq_d = nc.dram_tensor("wq", [D, D], mybir.dt.bfloat16, kind="ExternalInput").ap()
    wk_d = nc.dram_tensor("wk", [D, D], mybir.dt.bfloat16, kind="ExternalInput").ap()
    wv_d = nc.dram_tensor("wv", [D, D], mybir.dt.bfloat16, kind="ExternalInput").ap()
    wo_d = nc.dram_tensor("wo", [2, D, D], F32R, kind="ExternalInput").ap()
    # rows: bq_eff, bk_eff, rbias
    vecs_d = nc.dram_tensor("vecs", [3, D], F32, kind="ExternalInput").ap()
    outT_d = nc.dram_tensor("outT", [D, QH], F32, kind="ExternalOutput").ap()

    with tile.TileContext(nc) as tc:
        consts = tc.alloc_tile_pool(name="consts", bufs=1)
        sbW = tc.alloc_tile_pool(name="sbW", bufs=1)
        sbBig = tc.alloc_tile_pool(name="sbBig", bufs=1)
        sbTmp = tc.alloc_tile_pool(name="sbTmp", bufs=3)

        ident = consts.tile([128, 128], F32)
        make_identity(nc, ident)
        nshift = consts.tile([128, 1], F32)
        nc.vector.memset(nshift, -SHIFT)
        epsc = consts.tile([128, 1], F32)
        nc.vector.memset(epsc, EPS)
        zeroc = consts.tile([128, 1], F32)
        nc.vector.memset(zeroc, 0.0)
        wsrc = consts.tile([128, 512], BF16)
        nc.vector.memset(wsrc, 0.5)
        wones = consts.tile([128, DH], BF16)
        nc.vector.memset(wones, 1.0)

        # ---- input DMAs ----
        wq_f = sbW.tile([D, D], BF16)
        wk_f = sbW.tile([D, D], BF16)
        wv_f = sbW.tile([D, D], BF16)
        wo_p = sbW.tile([D, 2, D], F32R)
        nc.scalar.dma_start(out=wq_f, in_=wq_d)
        nc.scalar.dma_start(out=wk_f, in_=wk_d)
        nc.scalar.dma_start(out=wv_f, in_=wv_d)
        nc.scalar.dma_start(out=wo_p, in_=wo_d.rearrange("g d e -> d g e"))
        smallT = sbW.tile([D, 3], F32)  # cols: bq_eff, bk_eff, rbias
        nc.scalar.dma_start(out=smallT, in_=vecs_d.rearrange("v d -> d v"))
        xt_sb = sbBig.tile([128, QH], F32)
        nc.sync.dma_start(out=xt_sb, in_=xt_d)

        xkv_sb = sbBig.tile([128, NT, 128], F32)
        xkv_r = xkv_d.rearrange("(p t) d -> p t d", t=NT)
        for c4 in range(4):
            nc.sync.dma_start(
                out=xkv_sb[:, c4 * 4 : (c4 + 1) * 4, :],
                in_=xkv_r[:, c4 * 4 : (c4 + 1) * 4, :],
            )

        ps_a = tc.alloc_tile_pool(name="ps_a", bufs=2, space="PSUM")

        # HAM warm-up chain (independent; fills PE during DVE/DMA setup)
        for _ in range(8):
            wps = ps_a.tile([128, 512], F32, name="wps", tag="a")
            nc.tensor.matmul(wps[0:DH, :], wones, wsrc, start=True, stop=True)

        bqe = smallT[:, 0:1]
        bke = smallT[:, 1:2]
        rbias = smallT[:, 2:3]


        # ---- LayerNorm + transposes + projections, pipelined with the
        # attention loop: prep block b covers s-tiles 4b..4b+3 (their LN,
        # transpose, K/Q projection chunk and V tiles); attention ktiles
        # 4b..4b+3 of chunk 0 only need blocks <= b, so emission interleaves
        # prep blocks with attention ktiles and the PE stream never waits on
        # the full setup.
        mv_all = sbBig.tile([128, NT, 2], F32)
        lnv = sbBig.tile([128, NT], F32)
        rs_all = sbBig.tile([128, NT], F32)
        bias2 = sbBig.tile([128, NT], F32)
        xn0_sb = sbBig.tile([128, NT, 128], F32)
        xkvT = sbBig.tile([128, S], BF16)  # xn0^T [d, s]
        kT = sbBig.tile([128, S], BF16)
        qT = sbBig.tile([128, QH], BF16)
        v_sb = sbBig.tile([128, NT, H, 64], BF16)
        nc.vector.memset(v_sb[:, :, :, 33:64], 0.0)
        nc.vector.memset(v_sb[:, :, :, 0:1], 1.0)
        msel_f = consts.tile([128, 128], F32)
        nc.vector.memset(msel_f, 0.0)
        nc.vector.memset(msel_f[0:1, 0:64], 1.0)
        nc.vector.memset(msel_f[64:65, 64:128], 1.0)
        msel = consts.tile([128, 128], F32R)
        nc.vector.tensor_copy(msel, msel_f)
        residT = sbBig.tile([128, QH], F32)  # x^T + resid_bias (query half)

        def prep_block(b4):
            for t in range(b4 * 4, b4 * 4 + 4):
                stats = sbTmp.tile([128, 6], F32, tag="st")
                nc.vector.bn_stats(stats, xkv_sb[:, t, :])
                nc.vector.bn_aggr(mv_all[:, t, :], stats)
            sl4 = slice(b4 * 4, b4 * 4 + 4)
            # rs = sqrt(1/(var+eps)): reciprocal exact on DVE, Sqrt on ACT
            nc.vector.tensor_scalar_add(lnv[:, sl4], mv_all[:, sl4, 1], epsc)
            nc.vector.reciprocal(bias2[:, sl4], lnv[:, sl4])
            nc.scalar.activation(
                rs_all[:, sl4], bias2[:, sl4], AF.Sqrt, bias=zeroc, scale=1.0
            )
            for t in range(b4 * 4, b4 * 4 + 4):
                nc.vector.tensor_scalar(
                    xn0_sb[:, t, :],
                    xkv_sb[:, t, :],
                    mv_all[:, t, 0:1],
                    rs_all[:, t : t + 1],
                    op0=OP.subtract,
                    op1=OP.mult,
                )
                tp = ps_a.tile([128, 128], F32, tag="a")
                nc.tensor.transpose(tp, xn0_sb[:, t, :], ident)
                nc.scalar.copy(xkvT[:, t * 128 : (t + 1) * 128], tp)
            c = b4
            pp = ps_a.tile([128, CHUNK], F32, tag="a")
            nc.tensor.matmul(
                pp, wk_f, xkvT[:, c * CHUNK : (c + 1) * CHUNK], start=True, stop=True
            )
            nc.vector.tensor_scalar_add(kT[:, c * CHUNK : (c + 1) * CHUNK], pp, bke)
            if c < NCH:
                pp = ps_a.tile([128, CHUNK], F32, tag="a")
                nc.tensor.matmul(
                    pp, wq_f, xkvT[:, c * CHUNK : (c + 1) * CHUNK],
                    start=True, stop=True,
                )
                nc.vector.tensor_scalar_add(
                    qT[:, c * CHUNK : (c + 1) * CHUNK], pp, bqe
                )
            for t in range(b4 * 4, b4 * 4 + 4):
                pp = ps_a.tile([128, 128], F32, tag="a")
                nc.tensor.matmul(
                    pp, xkvT[:, t * 128 : (t + 1) * 128], wv_f, start=True, stop=True
                )
                nc.scalar.copy(
                    v_sb[:, t, :, 1:33],
                    pp.rearrange("p (h d) -> p h d", h=4),
                )

        def resid_block(ts_range):
            t0, t1 = ts_range[0], ts_range[-1] + 1
            nc.vector.tensor_scalar_add(
                residT[:, t0 * 128 : t1 * 128], xt_sb[:, t0 * 128 : t1 * 128], rbias
            )

        # ---- attention (interleaved with prep blocks) ----
        ps_e = tc.alloc_tile_pool(name="ps_e", bufs=1, space="PSUM")
        pPool = tc.alloc_tile_pool(name="pPool", bufs=6)

        ctx_ps = {}
        tail_state = {}

        def attn_scores(qc, kt):
            q0 = qc * CHUNK
            k0 = kt * 128
            p_sb = [None, None]
            for g, heads in enumerate(GROUPS):
                sp = ps_e.tile([128, 2 * CHUNK], F32, name=f"s{g}", tag="s", bufs=2)
                for i, h in enumerate(heads):
                    nc.tensor.matmul(
                        sp[:, i * CHUNK : (i + 1) * CHUNK],
                        kT[h * DH : (h + 1) * DH, k0 : k0 + 128],
                        qT[h * DH : (h + 1) * DH, q0 : q0 + CHUNK],
                        start=True,
                        stop=True,
                        tile_position=(h * DH, 0),
                    )
                if g == 0:
                    pA = pPool.tile([128, 2 * CHUNK], BF16, tag="p")
                    nc.scalar.activation(pA, sp, AF.Exp, bias=nshift, scale=1.0)
                    p_sb[0] = pA
                else:
                    pB = pPool.tile([128, 2 * CHUNK], I16, tag="p")
                    nc.vector.tensor_scalar(pB, sp, SA, SB, op0=OP.mult, op1=OP.add)
                    p_sb[1] = pB.bitcast(BF16)
            return p_sb

        def attn_ctxden(qc, kt, p_sb):
            first, last = kt == 0, kt == NKT - 1
            for g, heads in enumerate(GROUPS):
                bank = "A" if g == 0 else "B"
                for i, h in enumerate(heads):
                    nc.tensor.matmul(
                        ctx_ps[bank][64 * i : 64 * i + 64, :],
                        v_sb[:, kt, h, :],
                        p_sb[g][:, i * CHUNK : (i + 1) * CHUNK],
                        start=first,
                        stop=last,
                        tile_position=(0, 64 * i),
                        skip_group_check=True,
                    )

        def tail_copy(qc, cps):
            st = {}
            for g in ("A", "B"):
                cs = sbTmp.tile([128, CHUNK], F32R, tag=f"cs{g}")
                nc.vector.tensor_copy(cs, cps[g])
                st[g] = cs
            tail_state[qc] = st

        def tail_div(qc, g):
            cs = tail_state[qc][g]
            dps = ps_a.tile([128, CHUNK], F32, name=f"dps{g}", tag="a")
            nc.tensor.matmul(dps, msel, cs, start=True, stop=True)
            dinv = sbTmp.tile([128, CHUNK], F32, tag=f"di{g}")
            nc.vector.reciprocal_approx_fast(dinv, dps)
            ctxn = sbTmp.tile([128, CHUNK], F32R, tag=f"cn{g}")
            nc.vector.tensor_mul(ctxn, cs, dinv)
            tail_state[qc][g + "n"] = ctxn

        def tail_out(qc):
            q0 = qc * CHUNK
            outp = ps_a.tile([128, CHUNK], F32, name="outp", tag="a")
            for gi, g in enumerate(("A", "B")):
                ctxn = tail_state[qc][g + "n"]
                nc.tensor.matmul(
                    outp,
                    wo_p[:, gi, :],
                    ctxn,
                    start=(gi == 0),
                    stop=(gi == 1),
                )
            fin = sbTmp.tile([128, CHUNK], F32, tag="fin")
            nc.vector.tensor_add(fin, outp, residT[:, q0 : q0 + CHUNK])
            nc.sync.dma_start(out=outT_d[:, q0 : q0 + CHUNK], in_=fin)

        prep_block(0)
        resid_block(range(0, 4))
        prep_block(1)
        resid_block(range(4, NQT))
        for _ in range(6):
            warm2 = ps_a.tile([128, CHUNK], F32, name="warm2", tag="a")
            nc.tensor.matmul(warm2[0:DH, :], wones, wsrc, start=True, stop=True)


        # chunk 0
        ctx_ps = {
            "A": ps_e.tile([128, CHUNK], F32, name="ctxA0", tag="ctx"),
            "B": ps_e.tile([128, CHUNK], F32, name="ctxB0", tag="den"),
        }
        pending = attn_scores(0, 0)
        for kt in range(NKT):
            if kt == 3:
                prep_block(2)
            elif kt == 7:
                prep_block(3)
            nxt = attn_scores(0, kt + 1) if kt + 1 < NKT else None
            attn_ctxden(0, kt, pending)
            pending = nxt
        ctx0 = ctx_ps
        tail_copy(0, ctx0)

        # chunk 1
        ctx_ps = {
            "A": ps_e.tile([128, CHUNK], F32, name="ctxA1", tag="ctx"),
            "B": ps_e.tile([128, CHUNK], F32, name="ctxB1", tag="den"),
        }
        for _ in range(4):
            warm3 = ps_a.tile([128, CHUNK], F32, name="warm3", tag="a")
            nc.tensor.matmul(warm3[0:DH, :], wones, wsrc, start=True, stop=True)
        pending = attn_scores(1, 0)
        for kt in range(NKT):
            if kt == 2:
                tail_div(0, "A")
            elif kt == 4:
                tail_div(0, "B")
            elif kt == 6:
                tail_out(0)
            nxt = attn_scores(1, kt + 1) if kt + 1 < NKT else None
            attn_ctxden(1, kt, pending)
            pending = nxt
        tail_copy(1, ctx_ps)
        tail_div(1, "A")
        tail_div(1, "B")
        tail_out(1)

        pPool.release()
        ps_e.release()
        ps_a.release()
        sbTmp.release()
        sbBig.release()
        sbW.release()
        consts.release()

    nc.compile()
    return nc


def _get_compiled():
    global _compiled
    if _compiled is None:
        _compiled = _build()
    return _compiled


# device position j <- host row (j%128)*16 + j//128
_DEV2HOST = (np.arange(S) % 128) * NT + np.arange(S) // 128
_HOSTPERM = np.empty(S, dtype=np.int64)
_HOSTPERM[_DEV2HOST] = np.arange(S)


def kernel(x, Wq, bq, Wk, bk, Wv, bv, gamma, beta, Wo, bo):
    bf16 = mybir.dt.np(mybir.dt.bfloat16)
    x = np.asarray(x, dtype=np.float32)
    Wq, Wk, Wv, Wo = (np.asarray(w, dtype=np.float64) for w in (Wq, Wk, Wv, Wo))
    gamma, beta = (np.asarray(v, dtype=np.float64) for v in (gamma, beta))
    bq, bk, bv, bo = (np.asarray(v, dtype=np.float64) for v in (bq, bk, bv, bo))
    wq = np.ascontiguousarray((Wq * gamma[:, None] * ISQ).astype(bf16))
    wk = np.ascontiguousarray((Wk * gamma[:, None]).astype(bf16))
    wv = np.ascontiguousarray((Wv * gamma[:, None]).astype(bf16))
    woAB = np.zeros((2, 128, 128), dtype=np.float64)
    woAB[0, 1:33] = Wo[0:32]; woAB[0, 65:97] = Wo[64:96]
    woAB[1, 1:33] = Wo[32:64]; woAB[1, 65:97] = Wo[96:128]
    wo = np.ascontiguousarray(woAB.astype(np.float32))
    bq_eff = (Wq.T @ beta + bq) * ISQ
    bk_eff = Wk.T @ beta + bk
    bv_eff = Wv.T @ beta + bv
    rbias = Wo.T @ bv_eff + bo
    vecs = np.ascontiguousarray(np.stack([bq_eff, bk_eff, rbias]).astype(np.float32))

    nc = _get_compiled()

    in_maps = []
    for c in range(N_CORES):
        b, half = c // 2, c % 2
        off = half * QH
        xroll = np.roll(x[b], -off, axis=0)
        xin = np.ascontiguousarray(xroll[_HOSTPERM])
        xt = np.ascontiguousarray(xroll[0:QH].T)
        in_maps.append(
            {"xkv": xin, "xt": xt, "wq": wq, "wk": wk, "wv": wv, "wo": wo, "vecs": vecs}
        )

    res = run_bass_kernel_spmd(nc, in_maps, core_ids=list(range(N_CORES)), trace=False)

    out = np.empty((B, S, D), dtype=np.float32)
    for c in range(N_CORES):
        b, half = c // 2, c % 2
        off = half * QH
        out[b, off : off + QH, :] = res.results[c]["outT"].T
    return out



# revision 15
# speedup vs baseline: 1.2357x; 1.1655x over previous
"""Multi-head self-attention (pre-LN, residual) Trainium2 Bass kernel, v2.

Problem: B=4, S=2048, D=128, H=4, Dh=32, fp32.
Sharding: 8 cores = 4 batches x 2 query-halves (1024 queries/core).
Each core receives its batch's full x, row-shuffled by the host so that
(a) the core's query half occupies device positions 0..1023 and (b) each
SBUF partition loads consecutive DRAM rows.

Dataflow ([feature, seq] layouts), per core:
  xn0^T --W--> Q^T,K^T [hd, s] bf16;  V [s, hd] bf16 with a per-head
                                      ones column appended (33 cols/head)
  S^T[k,q] = K^T.T @ Q^T   4 heads row-tiled (K=32 at rows h*32)
  P_A = exp(S^T - 8)       heads {0,2} on ACT (one [128,1024] op)
  P_B = schraudolph(S^T-8) heads {1,3} on DVE (per-head [128,512] ops:
                           int16(x*SA+SB) bits == bf16 exp)
  ctx+den fused: M=33 col-tiled matmuls, bankA={h0@0,h2@64},
                 bankB={h1@0,h3@64}; row 32/96 of each bank = den
  deninv: K=1 ones-matmul broadcasts den rows to [128,512], DVE fast
          reciprocal, multiply, then 4 K=32 row-positioned Wo matmuls
          accumulate the output projection (junk rows never read)
  out^T = Wo.T @ ctxn + (x^T + rbias)

Host folds gamma/beta/biases/ISQ into the projection weights (numpy),
permutes Wo rows to the 2-bank ctx layout, and ships x^T for the
residual, so the device does no weight prep.  LN rsqrt runs on DVE
(quake seed + 2 Newton steps) so ACT keeps one table set (exp) loaded.

PSUM (8 banks): sA scores [128,1024] x2 bufs = 4, sB0/sB1 per-head
[128,512] = 2, ctxA/ctxB = 2.  Prep and tail psum tiles ride the sA
ring (2-buf rotation absorbs single insertions); prep is dripped one
psum tile per attention iteration; chunk-0's tail overlaps chunk-1.
"""

import sys

if "/opt/trn_rl_repo" not in sys.path:
    sys.path.insert(0, "/opt/trn_rl_repo")

import numpy as np

import concourse.bacc as bacc
import concourse.tile as tile
import concourse.mybir as mybir
from concourse.bass_utils import run_bass_kernel_spmd
from concourse.masks import make_identity

F32 = mybir.dt.float32
F32R = mybir.dt.float32r
BF16 = mybir.dt.bfloat16
I16 = mybir.dt.int16
I32 = mybir.dt.int32
AF = mybir.ActivationFunctionType
OP = mybir.AluOpType

B, S, D = 4, 2048, 128
H, DH = 4, 32
N_CORES = 8
QH = S // 2  # queries per core
NT = S // 128  # 16 s-tiles
CHUNK = 512
NCH = QH // CHUNK  # q-chunks per core (2)
NKT = S // 128  # 16 k-tiles
EPS = 1e-6
SHIFT = 8.0
ISQ = 1.0 / np.sqrt(np.float32(DH))
# Schraudolph bf16 exp: int16(x*SA + SB).bits == bf16(exp(x - SHIFT))
SA = float(128.0 / np.log(2.0))
SB = float(127.0 * 128.0 - 0.0579 * 128.0 - SHIFT * 128.0 / np.log(2.0))
QK3 = 0x5F3759DF  # quake rsqrt seed

_compiled = None


def _build():
    nc = bacc.Bacc(
        "TRN2",
        target_bir_lowering=False,
        debug=False,
        enable_asserts=False,
        num_devices=N_CORES,
    )

    xkv_d = nc.dram_tensor("xkv", [S, D], F32, kind="ExternalInput").ap()
    xt_d = nc.dram_tensor("xt", [D, QH], F32, kind="ExternalInput").ap()
    wq_d = nc.dram_tensor("wq", [D, D], BF16, kind="ExternalInput").ap()
    wk_d = nc.dram_tensor("wk", [D, D], BF16, kind="ExternalInput").ap()
    wv_d = nc.dram_tensor("wv", [D, D], BF16, kind="ExternalInput").ap()
    # woAB[0] rows {0-31: h0, 64-95: h2}; woAB[1] rows {0-31: h1, 64-95: h3}
    woAB_d = nc.dram_tensor("woAB", [2, D, D], F32R, kind="ExternalInput").ap()
    # rows: bq_eff, bk_eff, rbias
    vecs_d = nc.dram_tensor("vecs", [3, D], F32, kind="ExternalInput").ap()
    outT_d = nc.dram_tensor("outT", [D, QH], F32, kind="ExternalOutput").ap()

    with tile.TileContext(nc) as tc:
        consts = tc.alloc_tile_pool(name="consts", bufs=1)
        sbW = tc.alloc_tile_pool(name="sbW", bufs=1)
        sbBig = tc.alloc_tile_pool(name="sbBig", bufs=1)
        sbTmp = tc.alloc_tile_pool(name="sbTmp", bufs=3)

        ident = consts.tile([128, 128], F32)
        make_identity(nc, ident)
        nshift = consts.tile([128, 1], F32)
        nc.vector.memset(nshift, -SHIFT)
        wsrc = consts.tile([128, 512], BF16)
        nc.vector.memset(wsrc, 0.5)
        wones = consts.tile([128, DH], BF16)
        nc.vector.memset(wones, 1.0)
        msel = consts.tile([128, 128], F32)
        nc.vector.memset(msel, 0.0)
        nc.vector.memset(msel[0:1, 0:64], 1.0)
        nc.vector.memset(msel[64:65, 64:128], 1.0)
        dummy = consts.tile([128, 1], F32)
        nc.vector.memset(dummy, 0.0)

        # ---- input DMAs ----
        wq_f = sbW.tile([D, D], BF16)
        wk_f = sbW.tile([D, D], BF16)
        wv_f = sbW.tile([D, D], BF16)
        wo_sb = sbW.tile([D, 2, D], F32R)
        nc.scalar.dma_start(out=wq_f, in_=wq_d)
        nc.scalar.dma_start(out=wk_f, in_=wk_d)
        nc.scalar.dma_start(out=wv_f, in_=wv_d)
        nc.scalar.dma_start(out=wo_sb, in_=woAB_d.rearrange("g d e -> d g e"))
        vecsT = sbW.tile([D, 3], F32)  # cols: bq_eff, bk_eff, rbias
        nc.scalar.dma_start(out=vecsT, in_=vecs_d.rearrange("v d -> d v"))
        residT = sbBig.tile([128, QH], F32)  # x^T + rbias (query half)
        xt_sb = sbBig.tile([128, QH], F32)
        nc.sync.dma_start(out=xt_sb, in_=xt_d)

        xkv_sb = sbBig.tile([128, NT, 128], F32)
        xkv_r = xkv_d.rearrange("(p t) d -> p t d", t=NT)
        for c4 in range(4):
            nc.sync.dma_start(
                out=xkv_sb[:, c4 * 4 : (c4 + 1) * 4, :],
                in_=xkv_r[:, c4 * 4 : (c4 + 1) * 4, :],
            )

        # ---- PSUM pool: sA 2x[128,1024]=4 banks, sB0/sB1/ctxA/ctxB 1 each ----
        ps = tc.alloc_tile_pool(name="ps", bufs=1, space="PSUM")

        def sA_tile(name):
            return ps.tile([128, 2 * CHUNK], F32, name=name, tag="sA", bufs=2)

        # force the exp table load early (hides the ~1.3us load in startup)
        warm_exp = sbTmp.tile([128, 1], F32, tag="we")
        nc.scalar.activation(warm_exp, dummy, AF.Exp, bias=nshift, scale=1.0)

        # HAM warm-up chain on the ctx banks (no consumers -> back-to-back)
        for _ in range(8):
            wps = ps.tile([128, CHUNK], F32, name="wps", tag="ctxA", bufs=1)
            nc.tensor.matmul(wps[0:DH, :], wones, wsrc, start=True, stop=True)

        bqe = vecsT[:, 0:1]
        bke = vecsT[:, 1:2]
        rbias = vecsT[:, 2:3]

        # residT = xt + rbias  (2x_2P SBUF mode)
        nc.vector.tensor_scalar_add(residT[:, 0:CHUNK], xt_sb[:, 0:CHUNK], rbias)
        nc.vector.tensor_scalar_add(
            residT[:, CHUNK:QH], xt_sb[:, CHUNK:QH], rbias
        )

        # ---- LayerNorm + transposes + projections ----
        mv_all = sbBig.tile([128, NT, 2], F32)
        rs_all = sbBig.tile([128, NT], F32)
        q1 = sbBig.tile([128, NT], F32)
        q2 = sbBig.tile([128, NT], F32)
        xn0_sb = sbBig.tile([128, NT, 128], F32)
        xkvT = sbBig.tile([128, S], BF16)  # xn0^T [d, s]
        kT = sbBig.tile([128, S], BF16)
        qT = sbBig.tile([128, QH], BF16)
        # V per head: 64 cols = [ones (den), 32 v-dims, 31 zeros]
        v_sb = sbBig.tile([128, NT, H, 64], BF16)
        nc.vector.memset(v_sb[:, :, :, 33:64], 0.0)
        nc.vector.memset(v_sb[:, :, :, 0:1], 1.0)

        def quake_rsqrt(sl4):
            # rs = 1/sqrt(var+eps), all on DVE (avoids ACT Sqrt table swap)
            va = mv_all[:, sl4, 1]
            a = rs_all[:, sl4]
            nc.vector.tensor_scalar_add(a, va, float(EPS))
            u = a.bitcast(I32)
            y = q1[:, sl4]
            yi = y.bitcast(I32)
            nc.vector.tensor_scalar(
                yi, u, 1, 0, op0=OP.logical_shift_right, op1=OP.bypass
            )
            nc.vector.tensor_scalar(yi, yi, -1, QK3, op0=OP.mult, op1=OP.add)
            t = q2[:, sl4]
            for it in range(2):
                # y = y * (1.5 - 0.5*a*y*y)
                nc.vector.tensor_mul(t, y, y)
                nc.vector.tensor_mul(t, t, a)
                nc.vector.tensor_scalar(t, t, -0.5, 1.5, op0=OP.mult, op1=OP.add)
                if it == 0:
                    nc.vector.tensor_mul(y, y, t)
                else:
                    nc.vector.tensor_mul(rs_all[:, sl4], y, t)

        def prep_ln(b4):
            # DVE-only part of a prep block (no psum)
            for t in range(b4 * 4, b4 * 4 + 4):
                stats = sbTmp.tile([128, 6], F32, tag="st")
                nc.vector.bn_stats(stats, xkv_sb[:, t, :])
                nc.vector.bn_aggr(mv_all[:, t, :], stats)
            sl4 = slice(b4 * 4, b4 * 4 + 4)
            quake_rsqrt(sl4)
            for t in range(b4 * 4, b4 * 4 + 4):
                nc.vector.tensor_scalar(
                    xn0_sb[:, t, :],
                    xkv_sb[:, t, :],
                    mv_all[:, t, 0:1],
                    rs_all[:, t : t + 1],
                    op0=OP.subtract,
                    op1=OP.mult,
                )

        def prep_tp(b4):
            # 4 transposes into one sA tile, one ACT copy -> xkvT (bf16)
            tp = sA_tile("tp")
            for i, t in enumerate(range(b4 * 4, b4 * 4 + 4)):
                nc.tensor.transpose(
                    tp[:, i * 128 : (i + 1) * 128], xn0_sb[:, t, :], ident
                )
            nc.scalar.copy(
                xkvT[:, b4 * 512 : (b4 + 1) * 512], tp[:, 0:512]
            )

        def prep_kq(b4):
            c = b4
            pp = sA_tile("pp")
            nc.tensor.matmul(
                pp[:, 0:CHUNK], wk_f, xkvT[:, c * CHUNK : (c + 1) * CHUNK],
                start=True, stop=True,
            )
            nc.vector.tensor_scalar_add(
                kT[:, c * CHUNK : (c + 1) * CHUNK], pp[:, 0:CHUNK], bke
            )
            if c < NCH:
                nc.tensor.matmul(
                    pp[:, CHUNK : 2 * CHUNK],
                    wq_f,
                    xkvT[:, c * CHUNK : (c + 1) * CHUNK],
                    start=True,
                    stop=True,
                )
                nc.vector.tensor_scalar_add(
                    qT[:, c * CHUNK : (c + 1) * CHUNK],
                    pp[:, CHUNK : 2 * CHUNK],
                    bqe,
                )

        def prep_v(b4):
            # 4 v-proj matmuls into one sA tile, one strided ACT copy
            vp = sA_tile("vp")
            for i, t in enumerate(range(b4 * 4, b4 * 4 + 4)):
                nc.tensor.matmul(
                    vp[:, i * 128 : (i + 1) * 128],
                    xkvT[:, t * 128 : (t + 1) * 128],
                    wv_f,
                    start=True,
                    stop=True,
                )
            sl4 = slice(b4 * 4, b4 * 4 + 4)
            vpv = vp[:, 0:512].rearrange("p (t h d) -> p t h d", t=4, h=4, d=32)
            nc.scalar.copy(v_sb[:, sl4, :, 1:33], vpv)

        # ---- attention ----
        pPool = tc.alloc_tile_pool(name="pPool", bufs=3)

        ctx_ps = {}

        def attn_scores(qc, kt):
            q0 = qc * CHUNK
            k0 = kt * 128
            # group A: heads 0,2 -> one [128,1024] psum tile, ACT exp
            sa = sA_tile("sa")
            for i, h in enumerate((0, 2)):
                nc.tensor.matmul(
                    sa[:, i * CHUNK : (i + 1) * CHUNK],
                    kT[h * DH : (h + 1) * DH, k0 : k0 + 128],
                    qT[h * DH : (h + 1) * DH, q0 : q0 + CHUNK],
                    start=True,
                    stop=True,
                    tile_position=(h * DH, 0),
                )
            pA = pPool.tile([128, 2 * CHUNK], BF16, tag="pA")
            nc.scalar.activation(pA, sa, AF.Exp, bias=nshift, scale=1.0)
            # group B: heads 1,3 -> per-head [128,512] psum tiles, DVE exp
            pBs = []
            for i, h in enumerate((1, 3)):
                sb = ps.tile(
                    [128, CHUNK], F32, name=f"sb{i}", tag=f"sB{i}", bufs=1
                )
                nc.tensor.matmul(
                    sb,
                    kT[h * DH : (h + 1) * DH, k0 : k0 + 128],
                    qT[h * DH : (h + 1) * DH, q0 : q0 + CHUNK],
                    start=True,
                    stop=True,
                    tile_position=(h * DH, 0),
                )
                pB = pPool.tile([128, CHUNK], I16, tag=f"pB{i}")
                nc.vector.tensor_scalar(pB, sb, SA, SB, op0=OP.mult, op1=OP.add)
                pBs.append(pB.bitcast(BF16))
            return pA, pBs

        def attn_ctx(kt, p_sb):
            pA, pBs = p_sb
            first, last = kt == 0, kt == NKT - 1
            # M=64 col-tiled: h at rows 64i..64i+31, row 64i+32 = den,
            # rows 64i+33..64i+63 = 0 (keeps the whole bank initialized)
            for i, h in enumerate((0, 2)):
                nc.tensor.matmul(
                    ctx_ps["A"][64 * i : 64 * i + 64, :],
                    v_sb[:, kt, h, :],
                    pA[:, i * CHUNK : (i + 1) * CHUNK],
                    start=first,
                    stop=last,
                    tile_position=(0, 64 * i),
                    skip_group_check=True,
                )
            for i, h in enumerate((1, 3)):
                nc.tensor.matmul(
                    ctx_ps["B"][64 * i : 64 * i + 64, :],
                    v_sb[:, kt, h, :],
                    pBs[i],
                    start=first,
                    stop=last,
                    tile_position=(0, 64 * i),
                    skip_group_check=True,
                )

        tail_state = {}

        def tail_copy(qc, cps):
            # psum->sbuf unload of both ctx banks (frees the banks)
            st = {}
            for g in ("A", "B"):
                cs = sbTmp.tile([128, CHUNK], F32, tag=f"cs{g}")
                nc.scalar.copy(cs, cps[g])
                st[g] = cs
            tail_state[qc] = st

        def tail_div(qc, g):
            # den broadcast via masked fp32 matmul, fast recip, multiply
            cs = tail_state[qc][g]
            dps = sA_tile(f"dps{g}")
            nc.tensor.matmul(dps[:, 0:CHUNK], msel, cs, start=True, stop=True)
            dinv = sbTmp.tile([128, CHUNK], F32, tag=f"di{g}")
            nc.vector.reciprocal_approx_fast(dinv, dps[:, 0:CHUNK])
            ctxn = sbTmp.tile([128, CHUNK], F32R, tag=f"cn{g}")
            nc.vector.tensor_mul(ctxn, cs, dinv)
            tail_state[qc][g + "n"] = ctxn

        def tail_out(qc):
            q0 = qc * CHUNK
            outp = sA_tile("outp")
            for gi, g in enumerate(("A", "B")):
                ctxn = tail_state[qc][g + "n"]
                nc.tensor.matmul(
                    outp[:, 0:CHUNK],
                    wo_sb[:, gi, :],
                    ctxn,
                    start=(gi == 0),
                    stop=(gi == 1),
                )
            fin = sbTmp.tile([128, CHUNK], F32, tag="fin")
            nc.vector.tensor_add(fin, outp[:, 0:CHUNK], residT[:, q0 : q0 + CHUNK])
            nc.sync.dma_start(out=outT_d[:, q0 : q0 + CHUNK], in_=fin)

        # ---- schedule ----
        prep_ln(0)
        prep_tp(0)
        prep_kq(0)
        prep_v(0)
        prep_ln(1)
        prep_tp(1)
        prep_kq(1)
        prep_v(1)

        # chunk 0; prep blocks 2,3 dripped one psum tile per iteration
        ctx_ps = {
            "A": ps.tile([128, CHUNK], F32, name="ctxA0", tag="ctxA", bufs=1),
            "B": ps.tile([128, CHUNK], F32, name="ctxB0", tag="ctxB", bufs=1),
        }
        drip = [
            lambda: prep_ln(2),
            lambda: prep_tp(2),
            lambda: prep_kq(2),
            lambda: prep_v(2),
            lambda: prep_ln(3),
            lambda: prep_tp(3),
            lambda: prep_kq(3),
            lambda: prep_v(3),
        ]
        pending = attn_scores(0, 0)
        for kt in range(NKT):
            if kt >= 1 and drip:
                drip.pop(0)()
            nxt = attn_scores(0, kt + 1) if kt + 1 < NKT else None
            attn_ctx(kt, pending)
            pending = nxt

        ctx0 = ctx_ps
        tail_copy(0, ctx0)

        # chunk 1; chunk-0 tail pieces interleaved
        ctx_ps = {
            "A": ps.tile([128, CHUNK], F32, name="ctxA1", tag="ctxA", bufs=1),
            "B": ps.tile([128, CHUNK], F32, name="ctxB1", tag="ctxB", bufs=1),
        }
        pending = attn_scores(1, 0)
        for kt in range(NKT):
            if kt == 2:
                tail_div(0, "A")
            elif kt == 4:
                tail_div(0, "B")
            elif kt == 6:
                tail_out(0)
            nxt = attn_scores(1, kt + 1) if kt + 1 < NKT else None
            attn_ctx(kt, pending)
            pending = nxt

        tail_copy(1, ctx_ps)
        tail_div(1, "A")
        tail_div(1, "B")
        tail_out(1)

        pPool.release()
        ps.release()
        sbTmp.release()
        sbBig.release()
        sbW.release()
        consts.release()

    nc.compile()
    return nc


def _get_compiled():
    global _compiled
    if _compiled is None:
        _compiled = _build()
    return _compiled


# device position j <- host row (j%128)*16 + j//128
_DEV2HOST = (np.arange(S) % 128) * NT + np.arange(S) // 128
_HOSTPERM = np.empty(S, dtype=np.int64)
_HOSTPERM[_DEV2HOST] = np.arange(S)


def kernel(x, Wq, bq, Wk, bk, Wv, bv, gamma, beta, Wo, bo):
    bf16 = mybir.dt.np(BF16)
    x = np.asarray(x, dtype=np.float32)
    Wq = np.asarray(Wq, dtype=np.float64)
    Wk = np.asarray(Wk, dtype=np.float64)
    Wv = np.asarray(Wv, dtype=np.float64)
    Wo = np.asarray(Wo, dtype=np.float64)
    gamma = np.asarray(gamma, dtype=np.float64)
    beta = np.asarray(beta, dtype=np.float64)
    bq = np.asarray(bq, dtype=np.float64)
    bk = np.asarray(bk, dtype=np.float64)
    bv = np.asarray(bv, dtype=np.float64)
    bo = np.asarray(bo, dtype=np.float64)

    # fold gamma (and ISQ into q) into the projections; beta into biases
    wq_f = np.ascontiguousarray((Wq * gamma[:, None] * ISQ).astype(bf16))
    wk_f = np.ascontiguousarray((Wk * gamma[:, None]).astype(bf16))
    wv_f = np.ascontiguousarray((Wv * gamma[:, None]).astype(bf16))
    bq_eff = (Wq.T @ beta + bq) * ISQ
    bk_eff = Wk.T @ beta + bk
    bv_eff = Wv.T @ beta + bv
    rbias = Wo.T @ bv_eff + bo

    # Wo rows permuted to the 2-bank ctx layout:
    # bank A holds h0 at partitions 0-31, h2 at 64-95; bank B h1/h3.
    woAB = np.zeros((2, D, D), dtype=np.float64)
    woAB[0, 1:33] = Wo[0 * DH : 1 * DH]
    woAB[0, 65:97] = Wo[2 * DH : 3 * DH]
    woAB[1, 1:33] = Wo[1 * DH : 2 * DH]
    woAB[1, 65:97] = Wo[3 * DH : 4 * DH]
    woAB = np.ascontiguousarray(woAB.astype(np.float32))

    vecs = np.ascontiguousarray(
        np.stack([bq_eff, bk_eff, rbias]).astype(np.float32)
    )

    nc = _get_compiled()

    in_maps = []
    for c in range(N_CORES):
        b, half = c // 2, c % 2
        off = half * QH
        xroll = np.roll(x[b], -off, axis=0)
        xin = np.ascontiguousarray(xroll[_HOSTPERM])
        xt = np.ascontiguousarray(xroll[0:QH].T)
        in_maps.append(
            {
                "xkv": xin,
                "xt": xt,
                "wq": wq_f,
                "wk": wk_f,
                "wv": wv_f,
                "woAB": woAB,
                "vecs": vecs,
            }
        )

    res = run_bass_kernel_spmd(nc, in_maps, core_ids=list(range(N_CORES)), trace=False)

    out = np.empty((B, S, D), dtype=np.float32)
    for c in range(N_CORES):
        b, half = c // 2, c % 2
        off = half * QH
        out[b, off : off + QH, :] = res.results[c]["outT"].T
    return out
